# revision 14
# baseline (speedup 1.0000x reference)
"""Trainium2 Bass kernel for nn_CausalSelfAttention_22016002359635.

Reference computation (B=64, T=512, D=1024, DK=16):
    q = x @ Wq + bq                       # [B,T,16]
    k = x @ Wk + bk                       # [B,T,16]
    v = x @ Wv + bv                       # [B,T,1024]
    k = where(padding_mask, -1e24, k)     # replace k rows at padded positions
    att = (q @ k^T) * 4.0                 # sqrt(16)
    att = where(causal_upper, -1e24, att)
    out = softmax(att, axis=-1) @ v

Sharding: data-parallel over batch, 8 batches per NeuronCore x 8 cores.

Device algorithm per (core, batch):
  - x^T tiles produced via PE transposes (128x128 blocks through PSUM).
  - One fused projection matmul computes [4*Wq | rowsum(4*Wq) | Wk]^T @ x^T,
    yielding q^T (pre-scaled by sqrt(dk)=4, exact power of two), a 4*qsum row,
    and k^T in one PSUM tile.  The padding mask is applied by zeroing padded
    columns of k^T (multiply by 0/1 mask) and adding a 17th contraction row
    (-1e24 at padded columns) against the 4*qsum row: this reproduces the
    reference's  att[t, padded s] = 4 * sum_d q[t,d] * (-1e24)  semantics.
  - Causal masking REPLACES (not adds) scores with exactly -1e24 to reproduce
    reference behaviour for rows whose entire prefix is padded (softmax then
    attends uniformly over future positions).  Diagonal blocks use
    copy_predicated; for t_tile 0 the full row range is materialized densely.
  - Softmax row max via reduce_max(negate), exp+rowsum fused on the scalar
    engine, normalization folded into the output scaling.
  - P^T via PE transposes; out = P^T.T @ v accumulated in PSUM.
"""

import os
import sys

for _p in ("/opt/trn_rl_repo", "/root/.axon_site/_ro/trn_rl_repo"):
    if os.path.isdir(_p) and _p not in sys.path:
        sys.path.insert(0, _p)

import numpy as np


def _ensure_ntff_hook():
    """Provide antenv.axon_hooks if the image lacks it, wiring the NTFF
    profiling hook to libaxon_pjrt.so so trace=True works under axon."""
    try:
        import antenv.axon_hooks  # noqa: F401
        return
    except ImportError:
        pass
    import types

    try:
        import antenv
    except ImportError:
        return
    holder = {"hook": None}
    try:
        sys.path.insert(0, "/root/.axon_site")
        from trn_agent_boot.trn_boot import _ntff_profile_via_ctypes
        so_path = "/opt/axon/libaxon_pjrt.so"
        if os.path.exists(so_path):
            holder["hook"] = _ntff_profile_via_ctypes(so_path)
    except Exception:
        pass
    mod = types.ModuleType("antenv.axon_hooks")
    mod.get_axon_ntff_profile_hook = lambda: holder["hook"]
    mod.set_axon_ntff_profile_hook = lambda h: holder.__setitem__("hook", h)
    sys.modules["antenv.axon_hooks"] = mod
    antenv.axon_hooks = mod


_ensure_ntff_hook()

import concourse.bass as bass
import concourse.tile as tile
from concourse import bacc, mybir
from concourse.bass import ds, ts
from concourse.bass_utils import run_bass_kernel_spmd
from concourse.masks import make_identity

F32 = mybir.dt.float32
F32R = mybir.dt.float32r
U8 = mybir.dt.uint8

B, T, D, DK = 64, 512, 1024, 16
NCORES = 8
NB = B // NCORES          # batches per core
NEG = -1e24               # the reference's -INF
NT = T // 128             # 4 t/s tiles per sequence
ND = D // 512             # 2 output column chunks
NK = D // 128             # 8 contraction chunks
QKM = 48                  # rows: 4*Wq (16) | 4*qsum (1) | pad | Wk at 32-47


def _build_program(nb=NB, use_f32r=True, dense_tiles=(True, False, False, False),
                   slot_dense=None, with_bias_qk=False, with_bias_v=False):
    """Build and compile the per-core Bass program (SPMD across 8 cores)."""
    nc = bacc.Bacc("TRN2", target_bir_lowering=False, debug=False,
                   num_devices=NCORES)

    xt8 = nc.dram_tensor("xt8", [nb, D, T], F32, kind="ExternalInput").ap()
    wqk = nc.dram_tensor("wqk", [D, QKM], F32, kind="ExternalInput").ap()
    wv = nc.dram_tensor("wv", [D, D], F32, kind="ExternalInput").ap()
    pmul = nc.dram_tensor("pmul", [nb, T], F32, kind="ExternalInput").ap()
    padd = nc.dram_tensor("padd", [nb, T], F32, kind="ExternalInput").ap()
    causal = nc.dram_tensor("causal", [128, 128], U8, kind="ExternalInput").ap()
    if with_bias_qk:
        bqk = nc.dram_tensor("bqk", [1, QKM], F32, kind="ExternalInput").ap()
    if with_bias_v:
        bv = nc.dram_tensor("bv", [1, D], F32, kind="ExternalInput").ap()
    out8 = nc.dram_tensor("out8", [nb, T, D], F32, kind="ExternalOutput").ap()

    MDT = F32R if use_f32r else F32
    if slot_dense is None:
        slot_dense = [True] * nb

    with tile.TileContext(nc) as tc:
        with (
            tc.tile_pool(name="consts", bufs=1) as consts,
            tc.tile_pool(name="xpool", bufs=2) as xpool,
            tc.tile_pool(name="xtpool", bufs=2) as xtpool,
            tc.tile_pool(name="vpool", bufs=2) as vpool,
            tc.tile_pool(name="qkpool", bufs=2) as qkpool,
            tc.tile_pool(name="smpool", bufs=8) as smpool,
            tc.tile_pool(name="expool", bufs=2) as expool,
            tc.tile_pool(name="extpool", bufs=2) as extpool,
            tc.tile_pool(name="opool", bufs=3) as opool,
            tc.tile_pool(name="pstr", bufs=2, space="PSUM") as pstr,
            tc.tile_pool(name="psqk", bufs=1, space="PSUM") as psqk,
            tc.tile_pool(name="psv", bufs=1, space="PSUM") as psv,
            tc.tile_pool(name="psatt", bufs=1, space="PSUM") as psatt,
            tc.tile_pool(name="psout", bufs=1, space="PSUM") as psout,
        ):
            # ---- resident constants ----
            wv_sb = consts.tile([128, NK, D], MDT)
            wv_r = wv.rearrange("(c p) d -> p c d", p=128).bitcast(MDT)
            wqk_sb = consts.tile([128, NK, QKM], F32)
            nc.sync.dma_start(out=wqk_sb, in_=wqk.rearrange("(c p) m -> p c m", p=128))
            causal_sb = consts.tile([128, 128], U8)
            nc.sync.dma_start(out=causal_sb, in_=causal)
            neginf_sb = consts.tile([128, 512], F32)
            nc.vector.memset(neginf_sb, NEG)
            ident = consts.tile([128, 128], F32)
            make_identity(nc, ident)
            if with_bias_qk:
                ones_sb = consts.tile([1, 512], F32)
                nc.vector.memset(ones_sb, 1.0)
                bqk_sb = consts.tile([1, QKM], F32)
                nc.sync.dma_start(out=bqk_sb, in_=bqk)
            if with_bias_v:
                ones_v = consts.tile([1, 512], MDT)
                nc.vector.memset(ones_v, 1.0)
            if with_bias_v:
                bv_sb = consts.tile([1, D], MDT)
                nc.sync.dma_start(out=bv_sb, in_=bv.bitcast(MDT))

            for b in range(nb):
                # ---- x^T comes pre-transposed from the host ----
                xT = xtpool.tile([128, NK, T], F32)
                xtb = xt8[b].rearrange("(c p) t -> p c t", p=128)
                xTr = (xtpool.tile([128, NK, T], MDT, name="xTr")
                       if use_f32r else xT)
                for k in range(NK):
                    nc.sync.dma_start(out=xT[:, k, :], in_=xtb[:, k, :])
                    if use_f32r:
                        nc.sync.dma_start(out=xTr[:, k, :],
                                          in_=xtb[:, k, :].bitcast(MDT))
                    if b == 0:
                        # issue Wv chunk loads after batch 0's x so the first
                        # qk matmul isn't stuck behind 4MB of weight DMA
                        nc.sync.dma_start(out=wv_sb[:, k, :], in_=wv_r[:, k, :])

                # ---- fused q/k/qsum projection ----
                # natural orientation (x^T stationary, wqk moving at N=48,
                # fp32-exact) then PE-transpose the [128,48] block: ~2x
                # cheaper than streaming x at fp32's 4 cycles/row.
                qkT_sb = qkpool.tile([QKM, T], F32, name="qkT_sb")
                for i in range(NT):
                    qknat = psqk.tile([128, QKM], F32, name="qknat")
                    for k in range(NK):
                        nc.tensor.matmul(
                            qknat, xT[:, k, ts(i, 128)], wqk_sb[:, k, :],
                            start=(k == 0),
                            stop=(k == NK - 1 and not with_bias_qk))
                    if with_bias_qk:
                        nc.tensor.matmul(qknat, ones_sb[:, 0:128], bqk_sb,
                                         start=False, stop=True)
                    qknat_sb = qkpool.tile([128, QKM], F32, name="qknat_sb")
                    nc.vector.tensor_copy(qknat_sb, qknat)
                    trq = pstr.tile([QKM, 128], F32, name="trq", tag="trp")
                    nc.tensor.transpose(trq, qknat_sb, ident)
                    nc.vector.tensor_copy(qkT_sb[:, ts(i, 128)], trq)

                kt = qkpool.tile([DK + 1, T], F32, name="kt")
                # broadcast pmul into partitions 32-47 so both tensor_mul
                # inputs share base partition 32 (SB-SB base-equality rule)
                pm = qkpool.tile([48, T], F32, name="pm")
                pmb = pmul[b:b + 1, :]
                nc.gpsimd.dma_start(
                    out=pm[32:48, :],
                    in_=bass.AP(tensor=pmb.tensor, offset=pmb.offset,
                                ap=[[0, DK]] + list(pmb.ap[1:])))
                nc.vector.tensor_mul(kt[0:DK, :], qkT_sb[32:48, :],
                                     pm[32:48, :])
                nc.sync.dma_start(out=kt[DK:DK + 1, :], in_=padd[b:b + 1, :])

                # ---- v = x @ Wv (+ bv) ----
                vsb = vpool.tile([128, NT, D], MDT)
                for i in range(NT):
                    vps = [psv.tile([128, 512], F32, name=f"vps{dj}")
                           for dj in range(ND)]
                    for k in range(NK):
                        for dj in range(ND):
                            nc.tensor.matmul(
                                vps[dj], xTr[:, k, ts(i, 128)],
                                wv_sb[:, k, ts(dj, 512)],
                                start=(k == 0),
                                stop=(k == NK - 1 and not with_bias_v))
                    for dj in range(ND):
                        if with_bias_v:
                            nc.tensor.matmul(vps[dj], ones_v[:, 0:128],
                                             bv_sb[:, ts(dj, 512)],
                                             start=False, stop=True)
                        nc.scalar.copy(vsb[:, i, ts(dj, 512)], vps[dj])

                # ---- attention row-tiles ----
                for i in range(NT):
                    nmm = (i + 1) * 128            # columns with real scores
                    dense_i = dense_tiles[i] and (i > 0 or slot_dense[b])
                    esm = T if dense_i else nmm   # softmax/PV domain
                    atps = psatt.tile([128, 512], F32, name="atps")
                    nc.tensor.matmul(atps[:, 0:nmm],
                                     qkT_sb[0:DK + 1, ts(i, 128)],
                                     kt[:, 0:nmm], start=True, stop=True)
                    # replace upper-triangular part of diagonal block with -1e24
                    nc.vector.copy_predicated(
                        atps[:, ts(i, 128)], causal_sb, neginf_sb[:, 0:128])
                    if esm > nmm:
                        # fill fully-masked future blocks with exactly -1e24
                        nc.vector.tensor_copy(
                            atps[:, nmm:esm], neginf_sb[:, 0:esm - nmm])
                    negmax = smpool.tile([128, 1], F32, name="negmax")
                    nc.vector.reduce_max(negmax, atps[:, 0:esm],
                                         axis=mybir.AxisListType.X, negate=True)
                    ex = expool.tile([128, 512], F32, name="ex")
                    rsum = smpool.tile([128, 1], F32, name="rsum")
                    nc.scalar.activation(
                        ex[:, 0:esm], atps[:, 0:esm],
                        mybir.ActivationFunctionType.Exp,
                        bias=negmax, accum_out=rsum)
                    rrs = smpool.tile([128, 1], F32, name="rrs")
                    nc.vector.reciprocal(rrs, rsum)

                    # P^T via PE transposes (one PSUM bank per t-tile)
                    nsc = esm // 128
                    trp2 = pstr.tile([128, 512], F32, name="trp")
                    for s in range(nsc):
                        nc.tensor.transpose(
                            trp2[:, ts(s, 128)], ex[:, ts(s, 128)], ident)
                    exT = extpool.tile([128, 512], MDT, name="exT")
                    nc.vector.tensor_copy(exT[:, 0:esm], trp2[:, 0:esm])

                    ops = [psout.tile([128, 512], F32, name=f"ops{dj}")
                           for dj in range(ND)]
                    for s in range(nsc):
                        for dj in range(ND):
                            nc.tensor.matmul(
                                ops[dj], exT[:, ts(s, 128)],
                                vsb[:, s, ts(dj, 512)],
                                start=(s == 0), stop=(s == nsc - 1))
                    for dj in range(ND):
                        osb = opool.tile([128, 512], F32, name="osb")
                        nc.scalar.activation(
                            osb, ops[dj], mybir.ActivationFunctionType.Copy,
                            bias=0.0, scale=rrs)
                        nc.sync.dma_start(
                            out=out8[b, ts(i, 128), ts(dj, 512)], in_=osb)

    nc.compile()
    return nc


def _host_prep(x, padding_mask, Wq, bq, Wk, bk, Wv, bv):
    """Precompute small host-side tensors (masks, fused qk weight)."""
    xt = np.ascontiguousarray(
        np.asarray(x, dtype=np.float32).transpose(0, 2, 1))
    Wv = np.ascontiguousarray(np.asarray(Wv), dtype=np.float32)
    Wq = np.asarray(Wq, dtype=np.float32)
    Wk = np.asarray(Wk, dtype=np.float32)
    bq = np.asarray(bq, dtype=np.float32)
    bk = np.asarray(bk, dtype=np.float32)
    bv = np.asarray(bv, dtype=np.float32)
    pmask = np.asarray(padding_mask).reshape(B, T).astype(bool)

    wq4 = (Wq.astype(np.float64) * 4.0).astype(np.float32)
    wqk = np.zeros((D, QKM), dtype=np.float32)
    wqk[:, 0:DK] = wq4
    wqk[:, DK] = wq4.astype(np.float64).sum(axis=1).astype(np.float32)
    wqk[:, 32:48] = Wk
    wqk = np.ascontiguousarray(wqk)

    pmul = np.where(pmask, np.float32(0.0), np.float32(1.0))
    padd = np.where(pmask, np.float32(NEG), np.float32(0.0))

    r = np.arange(128)
    causal = (r[None, :] > r[:, None]).astype(np.uint8)
    causal = np.ascontiguousarray(causal)

    bq4 = (bq.astype(np.float64) * 4.0).astype(np.float32)
    bqk = np.zeros((1, QKM), dtype=np.float32)
    bqk[0, 0:DK] = bq4
    bqk[0, DK] = bq4.astype(np.float64).sum()
    bqk[0, 32:48] = bk
    with_bias_qk = bool(np.any(bq != 0) or np.any(bk != 0))
    with_bias_v = bool(np.any(bv != 0))

    # a t-tile needs the dense (full row range) path iff some row in it can
    # have its entire prefix padded (then the reference's softmax max comes
    # from the causal -1e24 region and mass spills onto future positions).
    prefix_all = np.cumprod(pmask, axis=1).astype(bool)   # [B, T]
    dense_tiles = tuple(
        bool(prefix_all[:, it * 128: (it + 1) * 128].any()) if it > 0 else True
        for it in range(NT))
    dense_b = prefix_all[:, 0]                            # tile-0 dense per batch
    # sort dense batches first and deal slot-major so whole slots are sparse
    order = np.argsort(~dense_b, kind="stable").astype(np.int64)
    slot_dense = [bool(dense_b[order[j * NCORES:(j + 1) * NCORES]].any())
                  for j in range(B // NCORES)]

    return dict(xt=xt, wqk=wqk, wv=Wv, pmul=pmul, padd=padd, causal=causal,
                order=order, slot_dense=slot_dense,
                bqk=np.ascontiguousarray(bqk),
                bv=np.ascontiguousarray(bv.reshape(1, D)),
                with_bias_qk=with_bias_qk, with_bias_v=with_bias_v,
                dense_tiles=dense_tiles)


def _in_maps(prep, nb=NB, ncores=NCORES):
    maps = []
    for c in range(ncores):
        idx = prep["order"][[j * ncores + c for j in range(nb)]]
        m = {
            "xt8": np.ascontiguousarray(prep["xt"][idx]),
            "wqk": prep["wqk"],
            "wv": prep["wv"],
            "pmul": np.ascontiguousarray(prep["pmul"][idx]),
            "padd": np.ascontiguousarray(prep["padd"][idx]),
            "causal": prep["causal"],
        }
        if prep["with_bias_qk"]:
            m["bqk"] = prep["bqk"]
        if prep["with_bias_v"]:
            m["bv"] = prep["bv"]
        maps.append(m)
    return maps


def run(inputs, use_f32r=True, trace=False, tmpdir=None):
    """Build + run on 8 NeuronCores; returns (full_output, BassKernelResults)."""
    prep = _host_prep(**inputs)
    nc = _build_program(nb=NB, use_f32r=use_f32r,
                        dense_tiles=prep["dense_tiles"],
                        slot_dense=prep["slot_dense"],
                        with_bias_qk=prep["with_bias_qk"],
                        with_bias_v=prep["with_bias_v"])
    maps = _in_maps(prep)
    res = run_bass_kernel_spmd(nc, maps, list(range(NCORES)),
                               trace=trace, tmpdir=tmpdir)
    out = np.empty((B, T, D), dtype=np.float32)
    for c in range(NCORES):
        idx = prep["order"][[j * NCORES + c for j in range(NB)]]
        out[idx] = res.results[c]["out8"]
    return out, res


def kernel(**inputs):
    out, _ = run(inputs, use_f32r=True)
    return out


# revision 15
# speedup vs baseline: 1.0430x; 1.0430x over previous
"""Trainium2 Bass kernel for nn_CausalSelfAttention_22016002359635.

Reference computation (B=64, T=512, D=1024, DK=16):
    q = x @ Wq + bq                       # [B,T,16]
    k = x @ Wk + bk                       # [B,T,16]
    v = x @ Wv + bv                       # [B,T,1024]
    k = where(padding_mask, -1e24, k)     # replace k rows at padded positions
    att = (q @ k^T) * 4.0                 # sqrt(16)
    att = where(causal_upper, -1e24, att)
    out = softmax(att, axis=-1) @ v

Sharding: data-parallel over batch, 8 batches per NeuronCore x 8 cores.

Device algorithm per (core, batch):
  - x^T tiles produced via PE transposes (128x128 blocks through PSUM).
  - One fused projection matmul computes [4*Wq | rowsum(4*Wq) | Wk]^T @ x^T,
    yielding q^T (pre-scaled by sqrt(dk)=4, exact power of two), a 4*qsum row,
    and k^T in one PSUM tile.  The padding mask is applied by zeroing padded
    columns of k^T (multiply by 0/1 mask) and adding a 17th contraction row
    (-1e24 at padded columns) against the 4*qsum row: this reproduces the
    reference's  att[t, padded s] = 4 * sum_d q[t,d] * (-1e24)  semantics.
  - Causal masking REPLACES (not adds) scores with exactly -1e24 to reproduce
    reference behaviour for rows whose entire prefix is padded (softmax then
    attends uniformly over future positions).  Diagonal blocks use
    copy_predicated; for t_tile 0 the full row range is materialized densely.
  - Softmax row max via reduce_max(negate), exp+rowsum fused on the scalar
    engine, normalization folded into the output scaling.
  - P^T via PE transposes; out = P^T.T @ v accumulated in PSUM.
"""

import os
import sys

for _p in ("/opt/trn_rl_repo", "/root/.axon_site/_ro/trn_rl_repo"):
    if os.path.isdir(_p) and _p not in sys.path:
        sys.path.insert(0, _p)

import numpy as np


def _ensure_ntff_hook():
    """Provide antenv.axon_hooks if the image lacks it, wiring the NTFF
    profiling hook to libaxon_pjrt.so so trace=True works under axon."""
    try:
        import antenv.axon_hooks  # noqa: F401
        return
    except ImportError:
        pass
    import types

    try:
        import antenv
    except ImportError:
        return
    holder = {"hook": None}
    try:
        sys.path.insert(0, "/root/.axon_site")
        from trn_agent_boot.trn_boot import _ntff_profile_via_ctypes
        so_path = "/opt/axon/libaxon_pjrt.so"
        if os.path.exists(so_path):
            holder["hook"] = _ntff_profile_via_ctypes(so_path)
    except Exception:
        pass
    mod = types.ModuleType("antenv.axon_hooks")
    mod.get_axon_ntff_profile_hook = lambda: holder["hook"]
    mod.set_axon_ntff_profile_hook = lambda h: holder.__setitem__("hook", h)
    sys.modules["antenv.axon_hooks"] = mod
    antenv.axon_hooks = mod


_ensure_ntff_hook()

import concourse.bass as bass
import concourse.tile as tile
from concourse import bacc, mybir
from concourse.bass import ds, ts
from concourse.bass_utils import run_bass_kernel_spmd
from concourse.masks import make_identity

F32 = mybir.dt.float32
F32R = mybir.dt.float32r
U8 = mybir.dt.uint8

B, T, D, DK = 64, 512, 1024, 16
NCORES = 8
NB = B // NCORES          # batches per core
NEG = -1e24               # the reference's -INF
NT = T // 128             # 4 t/s tiles per sequence
ND = D // 512             # 2 output column chunks
NK = D // 128             # 8 contraction chunks
QKM = 48                  # rows: 4*Wq (16) | 4*qsum (1) | pad | Wk at 32-47


def _build_program(nb=NB, use_f32r=True, dense_tiles=(True, False, False, False),
                   slot_dense=None, with_bias_qk=False, with_bias_v=False):
    """Build and compile the per-core Bass program (SPMD across 8 cores)."""
    nc = bacc.Bacc("TRN2", target_bir_lowering=False, debug=False,
                   num_devices=NCORES)

    xt8 = nc.dram_tensor("xt8", [nb, D, T], F32, kind="ExternalInput").ap()
    wqk = nc.dram_tensor("wqk", [D, QKM], F32, kind="ExternalInput").ap()
    wv = nc.dram_tensor("wv", [D, D], F32, kind="ExternalInput").ap()
    pmul = nc.dram_tensor("pmul", [nb, T], F32, kind="ExternalInput").ap()
    padd = nc.dram_tensor("padd", [nb, T], F32, kind="ExternalInput").ap()
    causal = nc.dram_tensor("causal", [128, 128], U8, kind="ExternalInput").ap()
    if with_bias_qk:
        bqk = nc.dram_tensor("bqk", [1, QKM], F32, kind="ExternalInput").ap()
    if with_bias_v:
        bv = nc.dram_tensor("bv", [1, D], F32, kind="ExternalInput").ap()
    out8 = nc.dram_tensor("out8", [nb, T, D], F32, kind="ExternalOutput").ap()

    MDT = F32R if use_f32r else F32
    if slot_dense is None:
        slot_dense = [True] * nb

    with tile.TileContext(nc) as tc:
        with (
            tc.tile_pool(name="consts", bufs=1) as consts,
            tc.tile_pool(name="xpool", bufs=2) as xpool,
            tc.tile_pool(name="xtpool", bufs=2) as xtpool,
            tc.tile_pool(name="vpool", bufs=2) as vpool,
            tc.tile_pool(name="qkpool", bufs=2) as qkpool,
            tc.tile_pool(name="smpool", bufs=8) as smpool,
            tc.tile_pool(name="expool", bufs=2) as expool,
            tc.tile_pool(name="extpool", bufs=2) as extpool,
            tc.tile_pool(name="opool", bufs=3) as opool,
            tc.tile_pool(name="pstr", bufs=2, space="PSUM") as pstr,
            tc.tile_pool(name="psqk", bufs=1, space="PSUM") as psqk,
            tc.tile_pool(name="psv", bufs=1, space="PSUM") as psv,
            tc.tile_pool(name="psatt", bufs=1, space="PSUM") as psatt,
            tc.tile_pool(name="psout", bufs=1, space="PSUM") as psout,
        ):
            # ---- resident constants ----
            wv_sb = consts.tile([128, NK, D], MDT)
            wv_r = wv.rearrange("(c p) d -> p c d", p=128).bitcast(MDT)
            wqk_sb = consts.tile([128, NK, QKM], F32)
            nc.sync.dma_start(out=wqk_sb, in_=wqk.rearrange("(c p) m -> p c m", p=128))
            causal_sb = consts.tile([128, 128], U8)
            nc.sync.dma_start(out=causal_sb, in_=causal)
            neginf_sb = consts.tile([128, 512], F32)
            nc.vector.memset(neginf_sb, NEG)
            ident = consts.tile([128, 128], F32)
            make_identity(nc, ident)
            if with_bias_qk:
                ones_sb = consts.tile([1, 512], F32)
                nc.vector.memset(ones_sb, 1.0)
                bqk_sb = consts.tile([1, QKM], F32)
                nc.sync.dma_start(out=bqk_sb, in_=bqk)
            if with_bias_v:
                ones_v = consts.tile([1, 512], MDT)
                nc.vector.memset(ones_v, 1.0)
            if with_bias_v:
                bv_sb = consts.tile([1, D], MDT)
                nc.sync.dma_start(out=bv_sb, in_=bv.bitcast(MDT))

            for b in range(nb):
                # ---- x^T comes pre-transposed from the host ----
                xT = xtpool.tile([128, NK, T], F32)
                xtb = xt8[b].rearrange("(c p) t -> p c t", p=128)
                xTr = (xtpool.tile([128, NK, T], MDT, name="xTr")
                       if use_f32r else xT)
                for k in range(NK):
                    nc.sync.dma_start(out=xT[:, k, :], in_=xtb[:, k, :])
                    if use_f32r:
                        nc.sync.dma_start(out=xTr[:, k, :],
                                          in_=xtb[:, k, :].bitcast(MDT))
                    if b == 0:
                        # issue Wv chunk loads after batch 0's x so the first
                        # qk matmul isn't stuck behind 4MB of weight DMA
                        nc.sync.dma_start(out=wv_sb[:, k, :], in_=wv_r[:, k, :])

                # ---- fused q/k/qsum projection: qkps[m, t] (fp32-exact) ----
                qkps = psqk.tile([QKM, T], F32, name="qkps")
                for k in range(NK):
                    nc.tensor.matmul(
                        qkps, wqk_sb[:, k, :], xT[:, k, :],
                        start=(k == 0), stop=(k == NK - 1 and not with_bias_qk))
                if with_bias_qk:
                    nc.tensor.matmul(qkps, bqk_sb, ones_sb,
                                     start=False, stop=True)

                qt = qkpool.tile([DK + 1, T], F32, name="qt")
                nc.vector.tensor_copy(qt, qkps[0:DK + 1, :])
                kt = qkpool.tile([DK + 1, T], F32, name="kt")
                pm = qkpool.tile([DK, T], F32, name="pm")
                pmb = pmul[b:b + 1, :]
                nc.gpsimd.dma_start(
                    out=pm,
                    in_=bass.AP(tensor=pmb.tensor, offset=pmb.offset,
                                ap=[[0, DK]] + list(pmb.ap[1:])))
                nc.vector.tensor_mul(kt[0:DK, :], qkps[32:48, :], pm)
                nc.sync.dma_start(out=kt[DK:DK + 1, :], in_=padd[b:b + 1, :])

                # ---- v = x @ Wv (+ bv) ----
                vsb = vpool.tile([128, NT, D], MDT)
                for i in range(NT):
                    vps = [psv.tile([128, 512], F32, name=f"vps{dj}")
                           for dj in range(ND)]
                    for k in range(NK):
                        for dj in range(ND):
                            nc.tensor.matmul(
                                vps[dj], xTr[:, k, ts(i, 128)],
                                wv_sb[:, k, ts(dj, 512)],
                                start=(k == 0),
                                stop=(k == NK - 1 and not with_bias_v))
                    for dj in range(ND):
                        if with_bias_v:
                            nc.tensor.matmul(vps[dj], ones_v[:, 0:128],
                                             bv_sb[:, ts(dj, 512)],
                                             start=False, stop=True)
                        nc.scalar.copy(vsb[:, i, ts(dj, 512)], vps[dj])

                # ---- attention row-tiles ----
                for i in range(NT):
                    nmm = (i + 1) * 128            # columns with real scores
                    dense_i = dense_tiles[i] and (i > 0 or slot_dense[b])
                    esm = T if dense_i else nmm   # softmax/PV domain
                    atps = psatt.tile([128, 512], F32, name="atps")
                    nc.tensor.matmul(atps[:, 0:nmm], qt[:, ts(i, 128)],
                                     kt[:, 0:nmm], start=True, stop=True)
                    # replace upper-triangular part of diagonal block with -1e24
                    nc.vector.copy_predicated(
                        atps[:, ts(i, 128)], causal_sb, neginf_sb[:, 0:128])
                    if esm > nmm:
                        # fill fully-masked future blocks with exactly -1e24
                        nc.vector.tensor_copy(
                            atps[:, nmm:esm], neginf_sb[:, 0:esm - nmm])
                    negmax = smpool.tile([128, 1], F32, name="negmax")
                    nc.vector.reduce_max(negmax, atps[:, 0:esm],
                                         axis=mybir.AxisListType.X, negate=True)
                    ex = expool.tile([128, 512], F32, name="ex")
                    rsum = smpool.tile([128, 1], F32, name="rsum")
                    nc.scalar.activation(
                        ex[:, 0:esm], atps[:, 0:esm],
                        mybir.ActivationFunctionType.Exp,
                        bias=negmax, accum_out=rsum)
                    rrs = smpool.tile([128, 1], F32, name="rrs")
                    nc.vector.reciprocal(rrs, rsum)

                    # P^T via PE transposes (one PSUM bank per t-tile)
                    nsc = esm // 128
                    trp2 = pstr.tile([128, 512], F32, name="trp")
                    for s in range(nsc):
                        nc.tensor.transpose(
                            trp2[:, ts(s, 128)], ex[:, ts(s, 128)], ident)
                    exT = extpool.tile([128, 512], MDT, name="exT")
                    nc.vector.tensor_copy(exT[:, 0:esm], trp2[:, 0:esm])

                    ops = [psout.tile([128, 512], F32, name=f"ops{dj}")
                           for dj in range(ND)]
                    for s in range(nsc):
                        for dj in range(ND):
                            nc.tensor.matmul(
                                ops[dj], exT[:, ts(s, 128)],
                                vsb[:, s, ts(dj, 512)],
                                start=(s == 0), stop=(s == nsc - 1))
                    for dj in range(ND):
                        osb = opool.tile([128, 512], F32, name="osb")
                        nc.scalar.activation(
                            osb, ops[dj], mybir.ActivationFunctionType.Copy,
                            bias=0.0, scale=rrs)
                        nc.sync.dma_start(
                            out=out8[b, ts(i, 128), ts(dj, 512)], in_=osb)

    nc.compile()
    return nc


def _host_prep(x, padding_mask, Wq, bq, Wk, bk, Wv, bv):
    """Precompute small host-side tensors (masks, fused qk weight)."""
    xt = np.ascontiguousarray(
        np.asarray(x, dtype=np.float32).transpose(0, 2, 1))
    Wv = np.ascontiguousarray(np.asarray(Wv), dtype=np.float32)
    Wq = np.asarray(Wq, dtype=np.float32)
    Wk = np.asarray(Wk, dtype=np.float32)
    bq = np.asarray(bq, dtype=np.float32)
    bk = np.asarray(bk, dtype=np.float32)
    bv = np.asarray(bv, dtype=np.float32)
    pmask = np.asarray(padding_mask).reshape(B, T).astype(bool)

    wq4 = (Wq.astype(np.float64) * 4.0).astype(np.float32)
    wqk = np.zeros((D, QKM), dtype=np.float32)
    wqk[:, 0:DK] = wq4
    wqk[:, DK] = wq4.astype(np.float64).sum(axis=1).astype(np.float32)
    wqk[:, 32:48] = Wk
    wqk = np.ascontiguousarray(wqk)

    pmul = np.where(pmask, np.float32(0.0), np.float32(1.0))
    padd = np.where(pmask, np.float32(NEG), np.float32(0.0))

    r = np.arange(128)
    causal = (r[None, :] > r[:, None]).astype(np.uint8)
    causal = np.ascontiguousarray(causal)

    bq4 = (bq.astype(np.float64) * 4.0).astype(np.float32)
    bqk = np.zeros((1, QKM), dtype=np.float32)
    bqk[0, 0:DK] = bq4
    bqk[0, DK] = bq4.astype(np.float64).sum()
    bqk[0, 32:48] = bk
    with_bias_qk = bool(np.any(bq != 0) or np.any(bk != 0))
    with_bias_v = bool(np.any(bv != 0))

    # a t-tile needs the dense (full row range) path iff some row in it can
    # have its entire prefix padded (then the reference's softmax max comes
    # from the causal -1e24 region and mass spills onto future positions).
    prefix_all = np.cumprod(pmask, axis=1).astype(bool)   # [B, T]
    dense_tiles = tuple(
        bool(prefix_all[:, it * 128: (it + 1) * 128].any()) if it > 0 else True
        for it in range(NT))
    dense_b = prefix_all[:, 0]                            # tile-0 dense per batch
    # sort dense batches first and deal slot-major so whole slots are sparse
    order = np.argsort(~dense_b, kind="stable").astype(np.int64)
    slot_dense = [bool(dense_b[order[j * NCORES:(j + 1) * NCORES]].any())
                  for j in range(B // NCORES)]

    return dict(xt=xt, wqk=wqk, wv=Wv, pmul=pmul, padd=padd, causal=causal,
                order=order, slot_dense=slot_dense,
                bqk=np.ascontiguousarray(bqk),
                bv=np.ascontiguousarray(bv.reshape(1, D)),
                with_bias_qk=with_bias_qk, with_bias_v=with_bias_v,
                dense_tiles=dense_tiles)


def _in_maps(prep, nb=NB, ncores=NCORES):
    maps = []
    for c in range(ncores):
        idx = prep["order"][[j * ncores + c for j in range(nb)]]
        m = {
            "xt8": np.ascontiguousarray(prep["xt"][idx]),
            "wqk": prep["wqk"],
            "wv": prep["wv"],
            "pmul": np.ascontiguousarray(prep["pmul"][idx]),
            "padd": np.ascontiguousarray(prep["padd"][idx]),
            "causal": prep["causal"],
        }
        if prep["with_bias_qk"]:
            m["bqk"] = prep["bqk"]
        if prep["with_bias_v"]:
            m["bv"] = prep["bv"]
        maps.append(m)
    return maps


def run(inputs, use_f32r=True, trace=False, tmpdir=None):
    """Build + run on 8 NeuronCores; returns (full_output, BassKernelResults)."""
    prep = _host_prep(**inputs)
    nc = _build_program(nb=NB, use_f32r=use_f32r,
                        dense_tiles=prep["dense_tiles"],
                        slot_dense=prep["slot_dense"],
                        with_bias_qk=prep["with_bias_qk"],
                        with_bias_v=prep["with_bias_v"])
    maps = _in_maps(prep)
    res = run_bass_kernel_spmd(nc, maps, list(range(NCORES)),
                               trace=trace, tmpdir=tmpdir)
    out = np.empty((B, T, D), dtype=np.float32)
    for c in range(NCORES):
        idx = prep["order"][[j * NCORES + c for j in range(NB)]]
        out[idx] = res.results[c]["out8"]
    return out, res


def kernel(**inputs):
    out, _ = run(inputs, use_f32r=True)
    return out


# revision 16
# speedup vs baseline: 1.0491x; 1.0059x over previous
"""Trainium2 Bass kernel for nn_CausalSelfAttention_22016002359635.

Reference computation (B=64, T=512, D=1024, DK=16):
    q = x @ Wq + bq                       # [B,T,16]
    k = x @ Wk + bk                       # [B,T,16]
    v = x @ Wv + bv                       # [B,T,1024]
    k = where(padding_mask, -1e24, k)     # replace k rows at padded positions
    att = (q @ k^T) * 4.0                 # sqrt(16)
    att = where(causal_upper, -1e24, att)
    out = softmax(att, axis=-1) @ v

Sharding: data-parallel over batch, 8 batches per NeuronCore x 8 cores.

Device algorithm per (core, batch):
  - x^T tiles produced via PE transposes (128x128 blocks through PSUM).
  - One fused projection matmul computes [4*Wq | rowsum(4*Wq) | Wk]^T @ x^T,
    yielding q^T (pre-scaled by sqrt(dk)=4, exact power of two), a 4*qsum row,
    and k^T in one PSUM tile.  The padding mask is applied by zeroing padded
    columns of k^T (multiply by 0/1 mask) and adding a 17th contraction row
    (-1e24 at padded columns) against the 4*qsum row: this reproduces the
    reference's  att[t, padded s] = 4 * sum_d q[t,d] * (-1e24)  semantics.
  - Causal masking REPLACES (not adds) scores with exactly -1e24 to reproduce
    reference behaviour for rows whose entire prefix is padded (softmax then
    attends uniformly over future positions).  Diagonal blocks use
    copy_predicated; for t_tile 0 the full row range is materialized densely.
  - Softmax row max via reduce_max(negate), exp+rowsum fused on the scalar
    engine, normalization folded into the output scaling.
  - P^T via PE transposes; out = P^T.T @ v accumulated in PSUM.
"""

import os
import sys

for _p in ("/opt/trn_rl_repo", "/root/.axon_site/_ro/trn_rl_repo"):
    if os.path.isdir(_p) and _p not in sys.path:
        sys.path.insert(0, _p)

import numpy as np


def _ensure_ntff_hook():
    """Provide antenv.axon_hooks if the image lacks it, wiring the NTFF
    profiling hook to libaxon_pjrt.so so trace=True works under axon."""
    try:
        import antenv.axon_hooks  # noqa: F401
        return
    except ImportError:
        pass
    import types

    try:
        import antenv
    except ImportError:
        return
    holder = {"hook": None}
    try:
        sys.path.insert(0, "/root/.axon_site")
        from trn_agent_boot.trn_boot import _ntff_profile_via_ctypes
        so_path = "/opt/axon/libaxon_pjrt.so"
        if os.path.exists(so_path):
            holder["hook"] = _ntff_profile_via_ctypes(so_path)
    except Exception:
        pass
    mod = types.ModuleType("antenv.axon_hooks")
    mod.get_axon_ntff_profile_hook = lambda: holder["hook"]
    mod.set_axon_ntff_profile_hook = lambda h: holder.__setitem__("hook", h)
    sys.modules["antenv.axon_hooks"] = mod
    antenv.axon_hooks = mod


_ensure_ntff_hook()

import concourse.bass as bass
import concourse.tile as tile
from concourse import bacc, mybir
from concourse.bass import ds, ts
from concourse.bass_utils import run_bass_kernel_spmd
from concourse.masks import make_identity

F32 = mybir.dt.float32
F32R = mybir.dt.float32r
U8 = mybir.dt.uint8

B, T, D, DK = 64, 512, 1024, 16
NCORES = 8
NB = B // NCORES          # batches per core
NEG = -1e24               # the reference's -INF
NT = T // 128             # 4 t/s tiles per sequence
ND = D // 512             # 2 output column chunks
NK = D // 128             # 8 contraction chunks
QKM = 48                  # rows: 4*Wq (16) | 4*qsum (1) | pad | Wk at 32-47


def _build_program(nb=NB, use_f32r=True, dense_tiles=(True, False, False, False),
                   slot_dense=None, with_bias_qk=False, with_bias_v=False):
    """Build and compile the per-core Bass program (SPMD across 8 cores)."""
    nc = bacc.Bacc("TRN2", target_bir_lowering=False, debug=False,
                   num_devices=NCORES)

    xt8 = nc.dram_tensor("xt8", [nb, D, T], F32, kind="ExternalInput").ap()
    wqk = nc.dram_tensor("wqk", [D, QKM], F32, kind="ExternalInput").ap()
    wv = nc.dram_tensor("wv", [D, D], F32, kind="ExternalInput").ap()
    pmul = nc.dram_tensor("pmul", [nb, T], F32, kind="ExternalInput").ap()
    padd = nc.dram_tensor("padd", [nb, T], F32, kind="ExternalInput").ap()
    causal = nc.dram_tensor("causal", [128, 128], U8, kind="ExternalInput").ap()
    if with_bias_qk:
        bqk = nc.dram_tensor("bqk", [1, QKM], F32, kind="ExternalInput").ap()
    if with_bias_v:
        bv = nc.dram_tensor("bv", [1, D], F32, kind="ExternalInput").ap()
    out8 = nc.dram_tensor("out8", [nb, T, D], F32, kind="ExternalOutput").ap()

    MDT = F32R if use_f32r else F32
    if slot_dense is None:
        slot_dense = [True] * nb

    with tile.TileContext(nc) as tc:
        with (
            tc.tile_pool(name="consts", bufs=1) as consts,
            tc.tile_pool(name="xpool", bufs=2) as xpool,
            tc.tile_pool(name="xtpool", bufs=2) as xtpool,
            tc.tile_pool(name="vpool", bufs=2) as vpool,
            tc.tile_pool(name="qkpool", bufs=2) as qkpool,
            tc.tile_pool(name="smpool", bufs=8) as smpool,
            tc.tile_pool(name="expool", bufs=2) as expool,
            tc.tile_pool(name="extpool", bufs=2) as extpool,
            tc.tile_pool(name="opool", bufs=3) as opool,
            tc.tile_pool(name="pstr", bufs=2, space="PSUM") as pstr,
            tc.tile_pool(name="psqk", bufs=1, space="PSUM") as psqk,
            tc.tile_pool(name="psv", bufs=1, space="PSUM") as psv,
            tc.tile_pool(name="psatt", bufs=1, space="PSUM") as psatt,
            tc.tile_pool(name="psout", bufs=1, space="PSUM") as psout,
        ):
            # ---- resident constants ----
            wv_sb = consts.tile([128, NK, D], MDT)
            wv_r = wv.rearrange("(c p) d -> p c d", p=128).bitcast(MDT)
            wqk_sb = consts.tile([128, NK, QKM], F32)
            nc.sync.dma_start(out=wqk_sb, in_=wqk.rearrange("(c p) m -> p c m", p=128))
            causal_sb = consts.tile([128, 128], U8)
            nc.sync.dma_start(out=causal_sb, in_=causal)
            neginf_sb = consts.tile([128, 512], F32)
            nc.vector.memset(neginf_sb, NEG)
            ident = consts.tile([128, 128], F32)
            make_identity(nc, ident)
            if with_bias_qk:
                ones_sb = consts.tile([1, 512], F32)
                nc.vector.memset(ones_sb, 1.0)
                bqk_sb = consts.tile([1, QKM], F32)
                nc.sync.dma_start(out=bqk_sb, in_=bqk)
            if with_bias_v:
                ones_v = consts.tile([1, 512], MDT)
                nc.vector.memset(ones_v, 1.0)
            if with_bias_v:
                bv_sb = consts.tile([1, D], MDT)
                nc.sync.dma_start(out=bv_sb, in_=bv.bitcast(MDT))

            for b in range(nb):
                # ---- x^T comes pre-transposed from the host ----
                xT = xtpool.tile([128, NK, T], F32)
                xtb = xt8[b].rearrange("(c p) t -> p c t", p=128)
                xTr = (xtpool.tile([128, NK, T], MDT, name="xTr")
                       if use_f32r else xT)
                for k in range(NK):
                    nc.sync.dma_start(out=xT[:, k, :], in_=xtb[:, k, :])
                    if use_f32r:
                        nc.sync.dma_start(out=xTr[:, k, :],
                                          in_=xtb[:, k, :].bitcast(MDT))
                    if b == 0:
                        # issue Wv chunk loads after batch 0's x so the first
                        # qk matmul isn't stuck behind 4MB of weight DMA
                        nc.sync.dma_start(out=wv_sb[:, k, :], in_=wv_r[:, k, :])

                # ---- fused q/k/qsum projection: qkps[m, t] (fp32-exact) ----
                qkps = psqk.tile([QKM, T], F32, name="qkps")
                for k in range(NK):
                    nc.tensor.matmul(
                        qkps, wqk_sb[:, k, :], xT[:, k, :],
                        start=(k == 0), stop=(k == NK - 1 and not with_bias_qk))
                if with_bias_qk:
                    nc.tensor.matmul(qkps, bqk_sb, ones_sb,
                                     start=False, stop=True)

                qt = qkpool.tile([DK + 1, T], F32, name="qt")
                nc.vector.tensor_copy(qt, qkps[0:DK + 1, :])
                kt = qkpool.tile([DK + 1, T], F32, name="kt")
                pm = qkpool.tile([DK, T], F32, name="pm")
                pmb = pmul[b:b + 1, :]
                nc.gpsimd.dma_start(
                    out=pm,
                    in_=bass.AP(tensor=pmb.tensor, offset=pmb.offset,
                                ap=[[0, DK]] + list(pmb.ap[1:])))
                nc.vector.tensor_mul(kt[0:DK, :], qkps[32:48, :], pm)
                nc.sync.dma_start(out=kt[DK:DK + 1, :], in_=padd[b:b + 1, :])

                # ---- v = x @ Wv (+ bv) ----
                vsb = vpool.tile([128, NT, D], MDT)
                for i in range(NT):
                    vps = [psv.tile([128, 512], F32, name=f"vps{dj}")
                           for dj in range(ND)]
                    for k in range(NK):
                        for dj in range(ND):
                            nc.tensor.matmul(
                                vps[dj], xTr[:, k, ts(i, 128)],
                                wv_sb[:, k, ts(dj, 512)],
                                start=(k == 0),
                                stop=(k == NK - 1 and not with_bias_v))
                    for dj in range(ND):
                        if with_bias_v:
                            nc.tensor.matmul(vps[dj], ones_v[:, 0:128],
                                             bv_sb[:, ts(dj, 512)],
                                             start=False, stop=True)
                        nc.scalar.copy(vsb[:, i, ts(dj, 512)], vps[dj])

                # ---- attention row-tiles ----
                for i in range(NT):
                    nmm = (i + 1) * 128            # columns with real scores
                    dense_i = dense_tiles[i] and (i > 0 or slot_dense[b])
                    esm = T if dense_i else nmm   # softmax/PV domain
                    atps = psatt.tile([128, 512], F32, name="atps")
                    nc.tensor.matmul(atps[:, 0:nmm], qt[:, ts(i, 128)],
                                     kt[:, 0:nmm], start=True, stop=True)
                    # replace upper-triangular part of diagonal block with -1e24
                    nc.vector.copy_predicated(
                        atps[:, ts(i, 128)], causal_sb, neginf_sb[:, 0:128])
                    if esm > nmm:
                        # fill fully-masked future blocks with exactly -1e24
                        nc.vector.tensor_copy(
                            atps[:, nmm:esm], neginf_sb[:, 0:esm - nmm])
                    negmax = smpool.tile([128, 1], F32, name="negmax")
                    nc.vector.reduce_max(negmax, atps[:, 0:esm],
                                         axis=mybir.AxisListType.X, negate=True)
                    ex = expool.tile([128, 512], F32, name="ex")
                    rsum = smpool.tile([128, 1], F32, name="rsum")
                    nc.scalar.activation(
                        ex[:, 0:esm], atps[:, 0:esm],
                        mybir.ActivationFunctionType.Exp,
                        bias=negmax, accum_out=rsum)
                    rrs = smpool.tile([128, 1], F32, name="rrs")
                    nc.vector.reciprocal(rrs, rsum)

                    # P^T via PE transposes (one PSUM bank per t-tile)
                    nsc = esm // 128
                    trp2 = pstr.tile([128, 512], F32, name="trp")
                    for s in range(nsc):
                        nc.tensor.transpose(
                            trp2[:, ts(s, 128)], ex[:, ts(s, 128)], ident)
                    exT = extpool.tile([128, 512], MDT, name="exT")
                    nc.vector.tensor_copy(exT[:, 0:esm], trp2[:, 0:esm])

                    ops = [psout.tile([128, 512], F32, name=f"ops{dj}")
                           for dj in range(ND)]
                    for s in range(nsc):
                        for dj in range(ND):
                            nc.tensor.matmul(
                                ops[dj], exT[:, ts(s, 128)],
                                vsb[:, s, ts(dj, 512)],
                                start=(s == 0), stop=(s == nsc - 1))
                    for dj in range(ND):
                        osb = opool.tile([128, 512], F32, name="osb")
                        nc.scalar.activation(
                            osb, ops[dj], mybir.ActivationFunctionType.Copy,
                            bias=0.0, scale=rrs)
                        nc.sync.dma_start(
                            out=out8[b, ts(i, 128), ts(dj, 512)], in_=osb)

    nc.compile()
    return nc


def _host_prep(x, padding_mask, Wq, bq, Wk, bk, Wv, bv):
    """Precompute small host-side tensors (masks, fused qk weight)."""
    xt = np.ascontiguousarray(
        np.asarray(x, dtype=np.float32).transpose(0, 2, 1))
    Wv = np.ascontiguousarray(np.asarray(Wv), dtype=np.float32)
    Wq = np.asarray(Wq, dtype=np.float32)
    Wk = np.asarray(Wk, dtype=np.float32)
    bq = np.asarray(bq, dtype=np.float32)
    bk = np.asarray(bk, dtype=np.float32)
    bv = np.asarray(bv, dtype=np.float32)
    pmask = np.asarray(padding_mask).reshape(B, T).astype(bool)

    wq4 = (Wq.astype(np.float64) * 4.0).astype(np.float32)
    wqk = np.zeros((D, QKM), dtype=np.float32)
    wqk[:, 0:DK] = wq4
    wqk[:, DK] = wq4.astype(np.float64).sum(axis=1).astype(np.float32)
    wqk[:, 32:48] = Wk
    wqk = np.ascontiguousarray(wqk)

    pmul = np.where(pmask, np.float32(0.0), np.float32(1.0))
    padd = np.where(pmask, np.float32(NEG), np.float32(0.0))

    r = np.arange(128)
    causal = (r[None, :] > r[:, None]).astype(np.uint8)
    causal = np.ascontiguousarray(causal)

    bq4 = (bq.astype(np.float64) * 4.0).astype(np.float32)
    bqk = np.zeros((1, QKM), dtype=np.float32)
    bqk[0, 0:DK] = bq4
    bqk[0, DK] = bq4.astype(np.float64).sum()
    bqk[0, 32:48] = bk
    with_bias_qk = bool(np.any(bq != 0) or np.any(bk != 0))
    with_bias_v = bool(np.any(bv != 0))

    # a t-tile needs the dense (full row range) path iff some row in it can
    # have its entire prefix padded (then the reference's softmax max comes
    # from the causal -1e24 region and mass spills onto future positions).
    prefix_all = np.cumprod(pmask, axis=1).astype(bool)   # [B, T]
    dense_tiles = tuple(
        bool(prefix_all[:, it * 128: (it + 1) * 128].any()) if it > 0 else True
        for it in range(NT))
    dense_b = prefix_all[:, 0]                            # tile-0 dense per batch
    # sort dense batches first and deal slot-major so whole slots are sparse
    order = np.argsort(~dense_b, kind="stable").astype(np.int64)
    slot_dense = [bool(dense_b[order[j * NCORES:(j + 1) * NCORES]].any())
                  for j in range(B // NCORES)]

    return dict(xt=xt, wqk=wqk, wv=Wv, pmul=pmul, padd=padd, causal=causal,
                order=order, slot_dense=slot_dense,
                bqk=np.ascontiguousarray(bqk),
                bv=np.ascontiguousarray(bv.reshape(1, D)),
                with_bias_qk=with_bias_qk, with_bias_v=with_bias_v,
                dense_tiles=dense_tiles)


def _in_maps(prep, nb=NB, ncores=NCORES):
    maps = []
    for c in range(ncores):
        idx = prep["order"][[j * ncores + c for j in range(nb)]]
        m = {
            "xt8": np.ascontiguousarray(prep["xt"][idx]),
            "wqk": prep["wqk"],
            "wv": prep["wv"],
            "pmul": np.ascontiguousarray(prep["pmul"][idx]),
            "padd": np.ascontiguousarray(prep["padd"][idx]),
            "causal": prep["causal"],
        }
        if prep["with_bias_qk"]:
            m["bqk"] = prep["bqk"]
        if prep["with_bias_v"]:
            m["bv"] = prep["bv"]
        maps.append(m)
    return maps


def run(inputs, use_f32r=True, trace=False, tmpdir=None):
    """Build + run on 8 NeuronCores; returns (full_output, BassKernelResults)."""
    prep = _host_prep(**inputs)
    nc = _build_program(nb=NB, use_f32r=use_f32r,
                        dense_tiles=prep["dense_tiles"],
                        slot_dense=prep["slot_dense"],
                        with_bias_qk=prep["with_bias_qk"],
                        with_bias_v=prep["with_bias_v"])
    maps = _in_maps(prep)
    try:
        res = run_bass_kernel_spmd(nc, maps, list(range(NCORES)),
                                   trace=trace, tmpdir=tmpdir)
    except Exception:
        # transient device errors (e.g. a wedged core from a prior run)
        # usually clear on retry
        res = run_bass_kernel_spmd(nc, maps, list(range(NCORES)),
                                   trace=trace, tmpdir=tmpdir)
    out = np.empty((B, T, D), dtype=np.float32)
    for c in range(NCORES):
        idx = prep["order"][[j * NCORES + c for j in range(NB)]]
        out[idx] = res.results[c]["out8"]
    return out, res


def kernel(**inputs):
    out, _ = run(inputs, use_f32r=True)
    return out
